# revision 17
# baseline (speedup 1.0000x reference)
"""Multi-head causal self-attention (B=4, N=2048, D=1024, H=16) on 8 TRN2 cores.

Sharding: 8 cores = 4 batches x 2 head-groups (8 heads / 512 dims each).

v3 schedule (qs-major): the outer loop walks query strips (groups g=0..3);
for each group all 4 head-pairs run their S^T/exp/PV units. The two heads of
a pair live on SBUF partitions 0-63 / 64-127, so their S^T matmuls (K=64
contraction) are emitted back-to-back and execute CONCURRENTLY in the two
row-halves of the PE array (row tiling, tile_position (0,0)/(64,0) inferred
from base partitions). The attention stream is scalar-engine(exp)-bound, so
PE filler work is interleaved between units at an adaptive rate:
  - QKV projection chunks for token strip g+1 (needed by group g+1),
  - O-projection chunks for query strip g-1,
  - per-pair softmax normalization (emitted as soon as a pair finishes).
Softmax denominators ride as a ones-column in V (PV row HD); reciprocals are
computed with the custom-DVE reciprocal_approx_fast straight out of PSUM and
broadcast across partitions with the GPSIMD partition_broadcast (no ACT
table switches - the scalar engine runs Exp only - and no DRAM round-trip).
Causal masking = skip blocks above the diagonal; the four diagonal-region
blocks per (pair, group) shrink 512/384/256/128 wide, exp'd at exact width,
with a precomputed 0/1 triangle multiplied onto the leading 128 columns.

Dtypes: scores fp32 PSUM -> exp -> bf16 P^T; V/P^T path bf16; attnT bf16;
O-partials stored bf16 (summed f32 host-side with the other head-group).
"""

import numpy as np
import ml_dtypes

import concourse.bass as bass
import concourse.tile as tile
from concourse import bacc, mybir
from concourse import bass_utils
from concourse._compat import with_exitstack
from concourse.bass import ts, ds

B, N, D, H, HD = 4, 2048, 1024, 16, 64
GROUPS = 2              # head groups (cores per batch)
DC = D // GROUPS        # 512 dims per core
HPC = H // GROUPS       # 8 heads per core
P = 128
QW = 512                # query strip width / matmul free dim
NDIN = D // P           # 8 contraction chunks for QKV
NPAIR = DC // P         # 4 head-pairs (dq strips) per core
NTT = N // P            # 16 token tiles
NTS = N // QW           # 4 token strips == query groups
NQB = QW // P           # 4 key blocks per token strip

F32 = mybir.dt.float32
BF16 = mybir.dt.bfloat16


def _emit(ctx, tc, xT, wq, wk, wv, wo, bq, bk, bv, masks, out):
    nc = tc.nc
    EXP = mybir.ActivationFunctionType.Exp

    const = ctx.enter_context(tc.tile_pool(name="const", bufs=1))
    p_mm = ctx.enter_context(tc.tile_pool(name="p_mm", bufs=2, space="PSUM"))
    p_st = ctx.enter_context(tc.tile_pool(name="p_st", bufs=2, space="PSUM"))
    p_pv = ctx.enter_context(tc.tile_pool(name="p_pv", bufs=1, space="PSUM"))
    p_pt = ctx.enter_context(tc.tile_pool(name="p_pt", bufs=5))
    p_sm = ctx.enter_context(tc.tile_pool(name="p_sm", bufs=2))
    p_osb = ctx.enter_context(tc.tile_pool(name="p_osb", bufs=3))

    # ---- weights / constants / x^T, need-ordered on the sync (HWDGE)
    # queue: hardware descriptor generation, so the gpsimd engine stays
    # free for the partition broadcasts and normalization multiplies
    wqp = wq.rearrange("(c p) f -> p c f", p=P)
    wkp = wk.rearrange("(c p) f -> p c f", p=P)
    xTp = xT.rearrange("(c p) n -> p c n", p=P)
    wqs = const.tile([P, NPAIR, NDIN, P], BF16)
    wks = const.tile([P, NPAIR, NDIN, P], BF16)
    xt = const.tile([P, NDIN, N], BF16)
    maskt = const.tile([P, P], BF16)
    bqt = const.tile([P, NPAIR], F32)
    bkt = const.tile([P, NPAIR], F32)
    wvt = const.tile([P, NDIN, DC], BF16)
    bvb = const.tile([P, DC], F32)
    wot = const.tile([P, NPAIR, D], BF16)
    nc.sync.dma_start(out=maskt, in_=masks)
    nc.sync.dma_start(out=bqt, in_=bq.rearrange("(s p) -> p s", p=P))
    nc.sync.dma_start(out=bkt, in_=bk.rearrange("(s p) -> p s", p=P))
    # token strip 0 + pair-0 weights per-chunk so the first matmuls of the
    # Q projection start as soon as chunk 0 lands
    for c in range(NDIN):
        nc.sync.dma_start(out=wqs[:, 0, c], in_=wqp[:, c, ts(0, P)])
        nc.sync.dma_start(out=xt[:, c, ts(0, QW)], in_=xTp[:, c, ts(0, QW)])
    nc.sync.dma_start(out=wks[:, 0], in_=wkp[:, :, ts(0, P)])
    nc.sync.dma_start(out=wvt, in_=wv.rearrange("(c p) f -> p c f", p=P))
    nc.sync.dma_start(out=bvb, in_=bv.unsqueeze(0).partition_broadcast(P))
    for s in range(1, NPAIR):
        nc.sync.dma_start(out=wqs[:, s], in_=wqp[:, :, ts(s, P)])
        nc.sync.dma_start(out=wks[:, s], in_=wkp[:, :, ts(s, P)])
    for t in range(1, NTS):
        nc.sync.dma_start(out=xt[:, :, ts(t, QW)], in_=xTp[:, :, ts(t, QW)])
    nc.sync.dma_start(out=wot, in_=wo.rearrange("(c p) f -> p c f", p=P))

    attnT = const.tile([P, NPAIR, N], BF16)           # unnormalized attn^T
    vplus = const.tile([P, NTT, HPC, HD + 1], BF16)   # V | ones column
    # memset on a bf16 matmul-input tile is invalid ISA; write the ones
    # column via a DVE copy from an f32 staging tile
    ones_f32 = const.tile([P, NTT * HPC], F32)
    nc.vector.memset(ones_f32, 1.0)
    nc.vector.tensor_copy(
        out=vplus[:, :, :, HD:HD + 1],
        in_=ones_f32.rearrange("p (a b) -> p a b", b=HPC).unsqueeze(3),
    )

    qts = const.tile([P, NPAIR, N], BF16)
    kts = const.tile([P, NPAIR, N], BF16)

    # ---- PE filler generators ----
    def proj_qk_steps(t, s):
        """Q and K projections of token strip t for pair strip s."""
        for which in range(2):  # 0 = Q, 1 = K
            wtile = wqs if which == 0 else wks
            btile = bqt if which == 0 else bkt
            dst = qts if which == 0 else kts
            ps = p_mm.tile([P, QW], F32, tag="mm", name="ps_proj")
            for c0 in range(0, NDIN, 2):
                def step(ps=ps, s=s, c0=c0, wtile=wtile, t=t):
                    for c in (c0, c0 + 1):
                        nc.tensor.matmul(
                            ps, lhsT=wtile[:, s, c, :],
                            rhs=xt[:, c, ts(t, QW)],
                            start=(c == 0), stop=(c == NDIN - 1),
                        )
                yield step
            def fin(ps=ps, s=s, t=t, btile=btile, dst=dst):
                if t <= 2:
                    # early groups: the vector engine runs hot while the
                    # scalar engine idles - do the bias-add there (Identity
                    # shares the Exp table set, so no ACT table switch)
                    nc.scalar.activation(
                        out=dst[:, s, ts(t, QW)], in_=ps,
                        func=mybir.ActivationFunctionType.Identity,
                        bias=btile[:, s:s + 1])
                else:
                    nc.vector.tensor_scalar_add(
                        out=dst[:, s, ts(t, QW)], in0=ps,
                        scalar1=btile[:, s:s + 1])
            yield fin

    def proj_v_steps(t):
        for tt in range(NQB * t, NQB * (t + 1)):
            psv = p_mm.tile([P, DC], F32, tag="mm", name="psv")
            for c0 in range(0, NDIN, 2):
                def step(psv=psv, tt=tt, c0=c0):
                    for c in (c0, c0 + 1):
                        nc.tensor.matmul(
                            psv, lhsT=xt[:, c, ts(tt, P)], rhs=wvt[:, c, :],
                            start=(c == 0), stop=(c == NDIN - 1),
                        )
                yield step
            def finv(psv=psv, tt=tt):
                nc.vector.tensor_add(
                    out=vplus[:, tt, :, 0:HD],
                    in0=psv.rearrange("p (h d) -> p h d", d=HD),
                    in1=bvb.rearrange("p (h d) -> p h d", d=HD),
                )
            yield finv

    def o_steps(b):
        """O-projection for query strip b (requires strip-b attnT normed)."""
        for tt in range(NQB * b, NQB * (b + 1)):
            osb = p_osb.tile([P, D], BF16, tag="osb", name="osb")
            for half in range(2):
                pso = p_mm.tile([P, QW], F32, tag="mm", name="pso")
                for c0 in range(0, NPAIR, 2):
                    def step(pso=pso, tt=tt, half=half, c0=c0):
                        for c in (c0, c0 + 1):
                            nc.tensor.matmul(
                                pso, lhsT=attnT[:, c, ts(tt, P)],
                                rhs=wot[:, c, ds(half * QW, QW)],
                                start=(c == 0), stop=(c == NPAIR - 1),
                            )
                    yield step
                def fino(pso=pso, osb=osb, tt=tt, half=half):
                    nc.vector.tensor_copy(
                        out=osb[:, ds(half * QW, QW)], in_=pso)
                    if half == 1:
                        nc.sync.dma_start(out=out[ts(tt, P), :], in_=osb)
                yield fino

    def norm_steps(g, pair, stg):
        """Softmax normalization of (group g, pair): reciprocal of the PV
        ones-row (staged to partition 0 by evict), partition-broadcast
        (both require base partition 0), then scale attnT."""
        recip = p_sm.tile([P, 2, QW], F32, tag="recip", name="recip")
        def rstep(recip=recip, stg=stg):
            nc.vector.reciprocal_approx_fast(
                out=recip[0:1, :, :], in_=stg[0:1, :, :])
        yield rstep
        rb = p_sm.tile([P, 2, QW], F32, tag="rb", bufs=3, name="rb")
        def bstep(rb=rb, recip=recip):
            nc.gpsimd.partition_broadcast(rb, recip[0:1, :, :])
        yield bstep
        for h2 in range(2):
            def mul(rb=rb, pair=pair, h2=h2, g=g):
                po = h2 * HD
                sl = attnT[po:po + HD, pair, ts(g, QW)]
                nc.vector.tensor_mul(out=sl, in0=sl, in1=rb[po:po + HD, h2, :])
            yield mul

    # ---- attention unit machinery ----
    def unit_list(g):
        units = [(kc, QW, 0) for kc in range(NQB * g)]
        for j in range(NQB):
            units.append((NQB * g + j, QW - j * P, j * P))
        return units

    def emit_s(g, pair, kc, w, qoff):
        pst = p_st.tile([P, 2, QW], F32, tag="st", name="pst")
        q0 = g * QW + qoff
        nc.tensor.matmul(
            pst[:, 0, 0:w],
            lhsT=kts[0:HD, pair, ts(kc, P)],
            rhs=qts[0:HD, pair, ds(q0, w)],
            start=True, stop=True,
        )
        nc.tensor.matmul(
            pst[:, 1, 0:w],
            lhsT=kts[HD:P, pair, ts(kc, P)],
            rhs=qts[HD:P, pair, ds(q0, w)],
            start=True, stop=True,
        )
        pt = p_pt.tile([P, 2, QW], BF16, tag="pt", name="pt")
        nc.scalar.activation(
            out=pt[:, :, 0:w], in_=pst[:, :, 0:w], func=EXP, scale=0.125)
        if kc >= NQB * g:  # diagonal block: triangle mask on leading 128
            nc.vector.tensor_mul(pt[:, 0, 0:P], pt[:, 0, 0:P], maskt)
            nc.vector.tensor_mul(pt[:, 1, 0:P], pt[:, 1, 0:P], maskt)
        return pt

    def emit_pv(g, pair, u, pt, pvps, nkc):
        kc, w, qoff = u
        for h2 in range(2):
            nc.tensor.matmul(
                pvps[h2][:, qoff:QW],
                lhsT=vplus[:, kc, 2 * pair + h2, :],
                rhs=pt[:, h2, 0:w],
                start=(kc == 0), stop=(kc == nkc - 1),
            )

    def evict(g, pair, pvps):
        """PSUM -> SBUF: attn values to attnT, ones-row (denominators) to
        partition 0 of a staging tile (releases the pvp banks promptly)."""
        stg = p_sm.tile([P, 2, QW], F32, tag="stg", name="stg")
        for h2 in range(2):
            po = h2 * HD
            nc.vector.tensor_copy(
                out=attnT[po:po + HD, pair, ts(g, QW)], in_=pvps[h2][0:HD, :])
            nc.vector.tensor_copy(
                out=stg[0:1, h2, :], in_=pvps[h2][HD:HD + 1, :])
        return stg

    # ---- adaptive filler queues ----
    # "must" work (projections + normalization) has to land within the
    # current group; "soft" work (O-projection chunks) is deferred freely
    # into later, scalar-engine-bound groups to balance PE load
    must = []
    soft = []
    acc = [0.0]

    def pull_units(remaining):
        """Pull an even share of must-work, topping up with soft work."""
        if remaining <= 0:
            k = len(must)
        else:
            acc[0] += len(must) / remaining + 0.75
            k = int(acc[0])
            acc[0] -= k
        for _ in range(k):
            if must:
                must.pop(0)()
            elif soft:
                soft.pop(0)()

    def drain():
        while must:
            must.pop(0)()

    # ---- main schedule ----
    # startup: pair-0 Q/K + V of token strip 0 land first; remaining
    # pairs' Q/K become filler so group-0 attention starts ASAP
    for f in proj_qk_steps(0, 0):
        f()
    for f in proj_v_steps(0):
        f()
    for s in range(1, NPAIR):
        must.extend(proj_qk_steps(0, s))

    for g in range(NTS):
        if g < NTS - 1:
            for s in range(NPAIR):
                must.extend(proj_qk_steps(g + 1, s))
            must.extend(proj_v_steps(g + 1))
        if g > 0:
            soft.extend(o_steps(g - 1))
        units = unit_list(g)
        nkc = len(units)
        rem_units = NPAIR * (nkc + 1)
        for pair in range(NPAIR):
            pvps = [
                p_pv.tile([HD + 1, QW], F32, tag="pvA", name="pvA"),
                p_pv.tile([HD + 1, QW], F32, tag="pvB", name="pvB"),
            ]
            # batches of 2 units: S,S,S,S -> fillers -> PV,PV,PV,PV keeps
            # the (64,128)->(128,128) PE mode switches at 2 per TWO units
            prev = None
            for i0 in range(0, nkc, 2):
                batch = units[i0:i0 + 2]
                pts = [emit_s(g, pair, *u) for u in batch]
                pull_units(rem_units)
                rem_units -= 1
                pull_units(rem_units)
                rem_units -= 1
                if prev is not None:
                    for (u, pt) in prev:
                        emit_pv(g, pair, u, pt, pvps, nkc)
                prev = list(zip(batch, pts))
            for (u, pt) in prev:
                emit_pv(g, pair, u, pt, pvps, nkc)
            stg = evict(g, pair, pvps)
            must.extend(norm_steps(g, pair, stg))
            pull_units(rem_units)
            rem_units -= 1
        if g < NTS - 1:
            # next group needs its projections landed; drain leftovers
            drain()

    # tail: remaining normalization + O work
    drain()
    while soft:
        soft.pop(0)()
    for f in o_steps(NTS - 1):
        f()


_emit_wrapped = with_exitstack(_emit)

_NC_CACHE = None


def _build():
    global _NC_CACHE
    if _NC_CACHE is not None:
        return _NC_CACHE
    nc = bacc.Bacc("TRN2", target_bir_lowering=False, debug=False)
    xT = nc.dram_tensor("xt", [D, N], BF16, kind="ExternalInput").ap()
    wq = nc.dram_tensor("wq", [D, DC], BF16, kind="ExternalInput").ap()
    wk = nc.dram_tensor("wk", [D, DC], BF16, kind="ExternalInput").ap()
    wv = nc.dram_tensor("wv", [D, DC], BF16, kind="ExternalInput").ap()
    wo = nc.dram_tensor("wo", [DC, D], BF16, kind="ExternalInput").ap()
    bq = nc.dram_tensor("bq", [DC], F32, kind="ExternalInput").ap()
    bk = nc.dram_tensor("bk", [DC], F32, kind="ExternalInput").ap()
    bv = nc.dram_tensor("bv", [DC], F32, kind="ExternalInput").ap()
    masks = nc.dram_tensor("masks", [P, P], BF16, kind="ExternalInput").ap()
    out = nc.dram_tensor("out", [N, D], BF16, kind="ExternalOutput").ap()
    with tile.TileContext(nc) as tc:
        _emit_wrapped(tc, xT, wq, wk, wv, wo, bq, bk, bv, masks, out)
    nc.compile()
    _NC_CACHE = nc
    return nc


def _make_masks():
    # triangular 0/1 tile for the diagonal blocks of S^T: key <= query kept
    return np.triu(np.ones((P, P), np.float32)).astype(ml_dtypes.bfloat16)


def _in_maps(x, Wq, bq, Wk, bk, Wv, bv, Wo):
    masks = _make_masks()
    maps = []
    for b in range(B):
        xt_b = np.ascontiguousarray(np.asarray(x[b]).T)
        for g in range(GROUPS):
            sl = slice(g * DC, (g + 1) * DC)
            bf = ml_dtypes.bfloat16
            maps.append({
                "xt": xt_b.astype(bf),
                "wq": np.ascontiguousarray(Wq[:, sl]).astype(bf),
                "wk": np.ascontiguousarray(Wk[:, sl]).astype(bf),
                "wv": np.ascontiguousarray(Wv[:, sl]).astype(bf),
                "wo": np.ascontiguousarray(Wo[sl, :]).astype(bf),
                "bq": np.ascontiguousarray(bq[sl]),
                "bk": np.ascontiguousarray(bk[sl]),
                "bv": np.ascontiguousarray(bv[sl]),
                "masks": masks,
            })
    return maps


def run(inputs, trace=False, tmpdir=None):
    """Build+run on 8 cores. Returns (out [B,N,D] f32, BassKernelResults)."""
    x = np.asarray(inputs["x"], np.float32)
    args = [np.asarray(inputs[k], np.float32) for k in
            ("Wq", "bq", "Wk", "bk", "Wv", "bv", "Wo")]
    bo = np.asarray(inputs["bo"], np.float32)
    nc = _build()
    maps = _in_maps(x, *args)
    if trace:
        bass_utils.upload_artifacts = lambda d: d
    res = bass_utils.run_bass_kernel_spmd(
        nc, maps, core_ids=list(range(8)), trace=trace, tmpdir=tmpdir)
    out = np.empty((B, N, D), np.float32)
    for b in range(B):
        out[b] = (res.results[2 * b]["out"].astype(np.float32)
                  + res.results[2 * b + 1]["out"].astype(np.float32) + bo)
    return out, res


def kernel(**inputs):
    out, _ = run(inputs)
    return out


# revision 18
# speedup vs baseline: 1.0073x; 1.0073x over previous
"""Multi-head causal self-attention (B=4, N=2048, D=1024, H=16) on 8 TRN2 cores.

Sharding: 8 cores = 4 batches x 2 head-groups (8 heads / 512 dims each).

v3 schedule (qs-major): the outer loop walks query strips (groups g=0..3);
for each group all 4 head-pairs run their S^T/exp/PV units. The two heads of
a pair live on SBUF partitions 0-63 / 64-127, so their S^T matmuls (K=64
contraction) are emitted back-to-back and execute CONCURRENTLY in the two
row-halves of the PE array (row tiling, tile_position (0,0)/(64,0) inferred
from base partitions). The attention stream is scalar-engine(exp)-bound, so
PE filler work is interleaved between units at an adaptive rate:
  - QKV projection chunks for token strip g+1 (needed by group g+1),
  - O-projection chunks for query strip g-1,
  - per-pair softmax normalization (emitted as soon as a pair finishes).
Softmax denominators ride as a ones-column in V (PV row HD); reciprocals are
computed with the custom-DVE reciprocal_approx_fast straight out of PSUM and
broadcast across partitions with the GPSIMD partition_broadcast (no ACT
table switches - the scalar engine runs Exp only - and no DRAM round-trip).
Causal masking = skip blocks above the diagonal; the four diagonal-region
blocks per (pair, group) shrink 512/384/256/128 wide, exp'd at exact width,
with a precomputed 0/1 triangle multiplied onto the leading 128 columns.

Dtypes: scores fp32 PSUM -> exp -> bf16 P^T; V/P^T path bf16; attnT bf16;
O-partials stored bf16 (summed f32 host-side with the other head-group).
"""

import numpy as np
import ml_dtypes

import concourse.bass as bass
import concourse.tile as tile
from concourse import bacc, mybir
from concourse import bass_utils
from concourse._compat import with_exitstack
from concourse.bass import ts, ds

B, N, D, H, HD = 4, 2048, 1024, 16, 64
GROUPS = 2              # head groups (cores per batch)
DC = D // GROUPS        # 512 dims per core
HPC = H // GROUPS       # 8 heads per core
P = 128
QW = 512                # query strip width / matmul free dim
NDIN = D // P           # 8 contraction chunks for QKV
NPAIR = DC // P         # 4 head-pairs (dq strips) per core
NTT = N // P            # 16 token tiles
NTS = N // QW           # 4 token strips == query groups
NQB = QW // P           # 4 key blocks per token strip

F32 = mybir.dt.float32
BF16 = mybir.dt.bfloat16


def _emit(ctx, tc, xT, wq, wk, wv, wo, bq, bk, bv, masks, out):
    nc = tc.nc
    EXP = mybir.ActivationFunctionType.Exp

    const = ctx.enter_context(tc.tile_pool(name="const", bufs=1))
    p_mm = ctx.enter_context(tc.tile_pool(name="p_mm", bufs=2, space="PSUM"))
    p_st = ctx.enter_context(tc.tile_pool(name="p_st", bufs=2, space="PSUM"))
    p_pv = ctx.enter_context(tc.tile_pool(name="p_pv", bufs=1, space="PSUM"))
    p_pt = ctx.enter_context(tc.tile_pool(name="p_pt", bufs=5))
    p_sm = ctx.enter_context(tc.tile_pool(name="p_sm", bufs=2))
    p_osb = ctx.enter_context(tc.tile_pool(name="p_osb", bufs=3))

    # ---- weights / constants / x^T, need-ordered on the sync (HWDGE)
    # queue: hardware descriptor generation, so the gpsimd engine stays
    # free for the partition broadcasts and normalization multiplies
    wqp = wq.rearrange("(c p) f -> p c f", p=P)
    wkp = wk.rearrange("(c p) f -> p c f", p=P)
    xTp = xT.rearrange("(c p) n -> p c n", p=P)
    wqs = const.tile([P, NPAIR, NDIN, P], BF16)
    wks = const.tile([P, NPAIR, NDIN, P], BF16)
    xt = const.tile([P, NDIN, N], BF16)
    maskt = const.tile([P, P], BF16)
    bqt = const.tile([P, NPAIR], F32)
    bkt = const.tile([P, NPAIR], F32)
    wvt = const.tile([P, NDIN, DC], BF16)
    bvb = const.tile([P, DC], F32)
    wot = const.tile([P, NPAIR, D], BF16)
    nc.sync.dma_start(out=maskt, in_=masks)
    nc.sync.dma_start(out=bqt, in_=bq.rearrange("(s p) -> p s", p=P))
    nc.sync.dma_start(out=bkt, in_=bk.rearrange("(s p) -> p s", p=P))
    # token strip 0 + pair-0 weights per-chunk so the first matmuls of the
    # Q projection start as soon as chunk 0 lands
    for c in range(NDIN):
        nc.sync.dma_start(out=wqs[:, 0, c], in_=wqp[:, c, ts(0, P)])
        nc.sync.dma_start(out=xt[:, c, ts(0, QW)], in_=xTp[:, c, ts(0, QW)])
    nc.sync.dma_start(out=wks[:, 0], in_=wkp[:, :, ts(0, P)])
    nc.sync.dma_start(out=wvt, in_=wv.rearrange("(c p) f -> p c f", p=P))
    nc.sync.dma_start(out=bvb, in_=bv.unsqueeze(0).partition_broadcast(P))
    for s in range(1, NPAIR):
        nc.sync.dma_start(out=wqs[:, s], in_=wqp[:, :, ts(s, P)])
        nc.sync.dma_start(out=wks[:, s], in_=wkp[:, :, ts(s, P)])
    for t in range(1, NTS):
        nc.sync.dma_start(out=xt[:, :, ts(t, QW)], in_=xTp[:, :, ts(t, QW)])
    nc.sync.dma_start(out=wot, in_=wo.rearrange("(c p) f -> p c f", p=P))

    attnT = const.tile([P, NPAIR, N], BF16)           # unnormalized attn^T
    vplus = const.tile([P, NTT, HPC, HD + 1], BF16)   # V | ones column
    # memset on a bf16 matmul-input tile is invalid ISA; write the ones
    # column via a DVE copy from an f32 staging tile
    ones_f32 = const.tile([P, NTT * HPC], F32)
    nc.vector.memset(ones_f32, 1.0)
    nc.vector.tensor_copy(
        out=vplus[:, :, :, HD:HD + 1],
        in_=ones_f32.rearrange("p (a b) -> p a b", b=HPC).unsqueeze(3),
    )

    qts = const.tile([P, NPAIR, N], BF16)
    kts = const.tile([P, NPAIR, N], BF16)

    # ---- PE filler generators ----
    def proj_qk_steps(t, s):
        """Q and K projections of token strip t for pair strip s."""
        for which in range(2):  # 0 = Q, 1 = K
            wtile = wqs if which == 0 else wks
            btile = bqt if which == 0 else bkt
            dst = qts if which == 0 else kts
            ps = p_mm.tile([P, QW], F32, tag="mm", name="ps_proj")
            for c0 in range(0, NDIN, 2):
                def step(ps=ps, s=s, c0=c0, wtile=wtile, t=t):
                    for c in (c0, c0 + 1):
                        nc.tensor.matmul(
                            ps, lhsT=wtile[:, s, c, :],
                            rhs=xt[:, c, ts(t, QW)],
                            start=(c == 0), stop=(c == NDIN - 1),
                        )
                yield step
            def fin(ps=ps, s=s, t=t, btile=btile, dst=dst):
                nc.vector.tensor_scalar_add(
                    out=dst[:, s, ts(t, QW)], in0=ps,
                    scalar1=btile[:, s:s + 1])
            yield fin

    def proj_v_steps(t):
        for tt in range(NQB * t, NQB * (t + 1)):
            psv = p_mm.tile([P, DC], F32, tag="mm", name="psv")
            for c0 in range(0, NDIN, 2):
                def step(psv=psv, tt=tt, c0=c0):
                    for c in (c0, c0 + 1):
                        nc.tensor.matmul(
                            psv, lhsT=xt[:, c, ts(tt, P)], rhs=wvt[:, c, :],
                            start=(c == 0), stop=(c == NDIN - 1),
                        )
                yield step
            def finv(psv=psv, tt=tt):
                nc.vector.tensor_add(
                    out=vplus[:, tt, :, 0:HD],
                    in0=psv.rearrange("p (h d) -> p h d", d=HD),
                    in1=bvb.rearrange("p (h d) -> p h d", d=HD),
                )
            yield finv

    def o_steps(b):
        """O-projection for query strip b (requires strip-b attnT normed)."""
        for tt in range(NQB * b, NQB * (b + 1)):
            osb = p_osb.tile([P, D], BF16, tag="osb", name="osb")
            for half in range(2):
                pso = p_mm.tile([P, QW], F32, tag="mm", name="pso")
                for c0 in range(0, NPAIR, 2):
                    def step(pso=pso, tt=tt, half=half, c0=c0):
                        for c in (c0, c0 + 1):
                            nc.tensor.matmul(
                                pso, lhsT=attnT[:, c, ts(tt, P)],
                                rhs=wot[:, c, ds(half * QW, QW)],
                                start=(c == 0), stop=(c == NPAIR - 1),
                            )
                    yield step
                def fino(pso=pso, osb=osb, tt=tt, half=half):
                    nc.vector.tensor_copy(
                        out=osb[:, ds(half * QW, QW)], in_=pso)
                    if half == 1:
                        nc.sync.dma_start(out=out[ts(tt, P), :], in_=osb)
                yield fino

    def norm_steps(g, pair, stg):
        """Softmax normalization of (group g, pair): reciprocal of the PV
        ones-row (staged to partition 0 by evict), partition-broadcast
        (both require base partition 0), then scale attnT."""
        recip = p_sm.tile([P, 2, QW], F32, tag="recip", name="recip")
        def rstep(recip=recip, stg=stg):
            nc.vector.reciprocal_approx_fast(
                out=recip[0:1, :, :], in_=stg[0:1, :, :])
        yield rstep
        rb = p_sm.tile([P, 2, QW], F32, tag="rb", bufs=3, name="rb")
        def bstep(rb=rb, recip=recip):
            nc.gpsimd.partition_broadcast(rb, recip[0:1, :, :])
        yield bstep
        for h2 in range(2):
            def mul(rb=rb, pair=pair, h2=h2, g=g):
                po = h2 * HD
                sl = attnT[po:po + HD, pair, ts(g, QW)]
                nc.vector.tensor_mul(out=sl, in0=sl, in1=rb[po:po + HD, h2, :])
            yield mul

    # ---- attention unit machinery ----
    def unit_list(g):
        units = [(kc, QW, 0) for kc in range(NQB * g)]
        for j in range(NQB):
            units.append((NQB * g + j, QW - j * P, j * P))
        return units

    def emit_s(g, pair, kc, w, qoff):
        pst = p_st.tile([P, 2, QW], F32, tag="st", name="pst")
        q0 = g * QW + qoff
        nc.tensor.matmul(
            pst[:, 0, 0:w],
            lhsT=kts[0:HD, pair, ts(kc, P)],
            rhs=qts[0:HD, pair, ds(q0, w)],
            start=True, stop=True,
        )
        nc.tensor.matmul(
            pst[:, 1, 0:w],
            lhsT=kts[HD:P, pair, ts(kc, P)],
            rhs=qts[HD:P, pair, ds(q0, w)],
            start=True, stop=True,
        )
        pt = p_pt.tile([P, 2, QW], BF16, tag="pt", name="pt")
        nc.scalar.activation(
            out=pt[:, :, 0:w], in_=pst[:, :, 0:w], func=EXP, scale=0.125)
        if kc >= NQB * g:  # diagonal block: triangle mask on leading 128
            nc.vector.tensor_mul(pt[:, 0, 0:P], pt[:, 0, 0:P], maskt)
            nc.vector.tensor_mul(pt[:, 1, 0:P], pt[:, 1, 0:P], maskt)
        return pt

    def emit_pv(g, pair, u, pt, pvps, nkc):
        kc, w, qoff = u
        for h2 in range(2):
            nc.tensor.matmul(
                pvps[h2][:, qoff:QW],
                lhsT=vplus[:, kc, 2 * pair + h2, :],
                rhs=pt[:, h2, 0:w],
                start=(kc == 0), stop=(kc == nkc - 1),
            )

    def evict(g, pair, pvps):
        """PSUM -> SBUF: attn values to attnT, ones-row (denominators) to
        partition 0 of a staging tile (releases the pvp banks promptly)."""
        stg = p_sm.tile([P, 2, QW], F32, tag="stg", name="stg")
        for h2 in range(2):
            po = h2 * HD
            nc.vector.tensor_copy(
                out=attnT[po:po + HD, pair, ts(g, QW)], in_=pvps[h2][0:HD, :])
            nc.vector.tensor_copy(
                out=stg[0:1, h2, :], in_=pvps[h2][HD:HD + 1, :])
        return stg

    # ---- adaptive filler queues ----
    # "must" work (projections + normalization) has to land within the
    # current group; "soft" work (O-projection chunks) is deferred freely
    # into later, scalar-engine-bound groups to balance PE load
    must = []
    soft = []
    acc = [0.0]

    def pull_units(remaining):
        """Pull an even share of must-work, topping up with soft work."""
        if remaining <= 0:
            k = len(must)
        else:
            acc[0] += len(must) / remaining + 0.75
            k = int(acc[0])
            acc[0] -= k
        for _ in range(k):
            if must:
                must.pop(0)()
            elif soft:
                soft.pop(0)()

    def drain():
        while must:
            must.pop(0)()

    # ---- main schedule ----
    # startup: pair-0 Q/K + V of token strip 0 land first; remaining
    # pairs' Q/K become filler so group-0 attention starts ASAP
    for f in proj_qk_steps(0, 0):
        f()
    for f in proj_v_steps(0):
        f()
    for s in range(1, NPAIR):
        must.extend(proj_qk_steps(0, s))

    for g in range(NTS):
        if g < NTS - 1:
            for s in range(NPAIR):
                must.extend(proj_qk_steps(g + 1, s))
            must.extend(proj_v_steps(g + 1))
        if g > 0:
            soft.extend(o_steps(g - 1))
        units = unit_list(g)
        nkc = len(units)
        rem_units = NPAIR * (nkc + 1)
        for pair in range(NPAIR):
            pvps = [
                p_pv.tile([HD + 1, QW], F32, tag="pvA", name="pvA"),
                p_pv.tile([HD + 1, QW], F32, tag="pvB", name="pvB"),
            ]
            # batches of 2 units: S,S,S,S -> fillers -> PV,PV,PV,PV keeps
            # the (64,128)->(128,128) PE mode switches at 2 per TWO units
            prev = None
            for i0 in range(0, nkc, 2):
                batch = units[i0:i0 + 2]
                pts = [emit_s(g, pair, *u) for u in batch]
                pull_units(rem_units)
                rem_units -= 1
                pull_units(rem_units)
                rem_units -= 1
                if prev is not None:
                    for (u, pt) in prev:
                        emit_pv(g, pair, u, pt, pvps, nkc)
                prev = list(zip(batch, pts))
            for (u, pt) in prev:
                emit_pv(g, pair, u, pt, pvps, nkc)
            stg = evict(g, pair, pvps)
            must.extend(norm_steps(g, pair, stg))
            pull_units(rem_units)
            rem_units -= 1
        if g < NTS - 1:
            # next group needs its projections landed; drain leftovers
            drain()

    # tail: remaining normalization + O work
    drain()
    while soft:
        soft.pop(0)()
    for f in o_steps(NTS - 1):
        f()


_emit_wrapped = with_exitstack(_emit)

_NC_CACHE = None


def _build():
    global _NC_CACHE
    if _NC_CACHE is not None:
        return _NC_CACHE
    nc = bacc.Bacc("TRN2", target_bir_lowering=False, debug=False)
    xT = nc.dram_tensor("xt", [D, N], BF16, kind="ExternalInput").ap()
    wq = nc.dram_tensor("wq", [D, DC], BF16, kind="ExternalInput").ap()
    wk = nc.dram_tensor("wk", [D, DC], BF16, kind="ExternalInput").ap()
    wv = nc.dram_tensor("wv", [D, DC], BF16, kind="ExternalInput").ap()
    wo = nc.dram_tensor("wo", [DC, D], BF16, kind="ExternalInput").ap()
    bq = nc.dram_tensor("bq", [DC], F32, kind="ExternalInput").ap()
    bk = nc.dram_tensor("bk", [DC], F32, kind="ExternalInput").ap()
    bv = nc.dram_tensor("bv", [DC], F32, kind="ExternalInput").ap()
    masks = nc.dram_tensor("masks", [P, P], BF16, kind="ExternalInput").ap()
    out = nc.dram_tensor("out", [N, D], BF16, kind="ExternalOutput").ap()
    with tile.TileContext(nc) as tc:
        _emit_wrapped(tc, xT, wq, wk, wv, wo, bq, bk, bv, masks, out)
    nc.compile()
    _NC_CACHE = nc
    return nc


def _make_masks():
    # triangular 0/1 tile for the diagonal blocks of S^T: key <= query kept
    return np.triu(np.ones((P, P), np.float32)).astype(ml_dtypes.bfloat16)


def _in_maps(x, Wq, bq, Wk, bk, Wv, bv, Wo):
    masks = _make_masks()
    maps = []
    for b in range(B):
        xt_b = np.ascontiguousarray(np.asarray(x[b]).T)
        for g in range(GROUPS):
            sl = slice(g * DC, (g + 1) * DC)
            bf = ml_dtypes.bfloat16
            maps.append({
                "xt": xt_b.astype(bf),
                "wq": np.ascontiguousarray(Wq[:, sl]).astype(bf),
                "wk": np.ascontiguousarray(Wk[:, sl]).astype(bf),
                "wv": np.ascontiguousarray(Wv[:, sl]).astype(bf),
                "wo": np.ascontiguousarray(Wo[sl, :]).astype(bf),
                "bq": np.ascontiguousarray(bq[sl]),
                "bk": np.ascontiguousarray(bk[sl]),
                "bv": np.ascontiguousarray(bv[sl]),
                "masks": masks,
            })
    return maps


def run(inputs, trace=False, tmpdir=None):
    """Build+run on 8 cores. Returns (out [B,N,D] f32, BassKernelResults)."""
    x = np.asarray(inputs["x"], np.float32)
    args = [np.asarray(inputs[k], np.float32) for k in
            ("Wq", "bq", "Wk", "bk", "Wv", "bv", "Wo")]
    bo = np.asarray(inputs["bo"], np.float32)
    nc = _build()
    maps = _in_maps(x, *args)
    if trace:
        bass_utils.upload_artifacts = lambda d: d
    res = bass_utils.run_bass_kernel_spmd(
        nc, maps, core_ids=list(range(8)), trace=trace, tmpdir=tmpdir)
    out = np.empty((B, N, D), np.float32)
    for b in range(B):
        out[b] = (res.results[2 * b]["out"].astype(np.float32)
                  + res.results[2 * b + 1]["out"].astype(np.float32) + bo)
    return out, res


def kernel(**inputs):
    out, _ = run(inputs)
    return out
